# revision 8
# baseline (speedup 1.0000x reference)
"""Collisionless (multi-hash) embedding lookup on 8 Trainium2 NeuronCores.

Strategy: data-parallel shard of the token batch across 8 cores; both
embedding tables are concatenated into one [2M, 64] table and replicated
to every core.  The two per-token hashed indices are computed on host
(exact uint32 arithmetic, matches the jax reference bit-for-bit), the
second one offset by NUM_EMB so a single indirect-DMA gather per block
fetches interleaved (table0-row, table1-row) pairs, which lands the final
[token, 128] layout directly in SBUF.  Per block: indirect gather ->
contiguous store, software-pipelined over NBUF SBUF buffers.

Raw bass (not Tile): the gather->store->gather dependency chain needs
sequencer wait_ge instructions; Tile attaches waits to the DMA descriptors
themselves and walrus' direct2d codegen only allows one wait per DMA.
"""

from contextlib import ExitStack

import numpy as np

import concourse.bass as bass
import concourse.mybir as mybir
from concourse.bass_utils import run_bass_kernel_spmd

N_CORES = 8
P = 128
SUB_DIM = 64
EMBED = 128
NUM_EMB = 1_000_000
SEED = 42
BATCH, SEQ = 4096, 200
TOK_TOTAL = BATCH * SEQ          # 819200
TOK_CORE = TOK_TOTAL // N_CORES  # 102400
TOK_PART = TOK_CORE // P         # 800 tokens per partition per core
G = 64                           # gathers per store group
NBUF = 3                         # SBUF staging buffers (pipeline depth)


def _hash_ids(ids_u32: np.ndarray, seed: int) -> np.ndarray:
    x = ids_u32 ^ np.uint32(seed)
    x = (x ^ (x >> np.uint32(16))) * np.uint32(0x7FEB352D)
    x = (x ^ (x >> np.uint32(15))) * np.uint32(0x846CA68B)
    x = x ^ (x >> np.uint32(16))
    return (x % np.uint32(NUM_EMB)).astype(np.int32)


def build_nc(
    tok_part: int = TOK_PART,
    g_size: int = G,
    num_rows: int = 2 * NUM_EMB,
    nbuf: int = NBUF,
):
    """Per-core program: out[p, t, :] = table[idx[p, 2t]] ++ table[idx[p, 2t+1]].

    One indirect DMA per index column d (the only encoding walrus handles:
    [P, 1] offsets, one row per partition, 2D dest).  Gather d lands in
    column slice d%g_size of a staging buffer; every g_size gathers are
    flushed to DRAM with one large store.
    """
    ncol = tok_part * 2
    ngrp = ncol // g_size
    assert ncol % g_size == 0
    nc = bass.Bass("TRN2", debug=False)
    idx = nc.dram_tensor(
        "idx", [P, ncol], mybir.dt.int32, kind="ExternalInput"
    ).ap()
    table = nc.dram_tensor(
        "table", [num_rows, SUB_DIM], mybir.dt.float32, kind="ExternalInput"
    ).ap()
    out = nc.dram_tensor(
        "out", [P, tok_part * EMBED], mybir.dt.float32, kind="ExternalOutput"
    ).ap()

    with (
        ExitStack() as stack,
        nc.Block() as block,
    ):
        idx_all = stack.enter_context(
            nc.sbuf_tensor("idx_all", [P, ncol], mybir.dt.int32)
        )
        sem_idx = stack.enter_context(nc.semaphore("sem_idx"))
        bufs = [
            stack.enter_context(
                nc.sbuf_tensor(f"big{i}", [P, g_size * SUB_DIM], mybir.dt.float32)
            )
            for i in range(nbuf)
        ]
        sem_g = [stack.enter_context(nc.semaphore(f"sem_g{i}")) for i in range(nbuf)]
        sem_s = [stack.enter_context(nc.semaphore(f"sem_s{i}")) for i in range(nbuf)]
        n_store = [len(range(i, ngrp, nbuf)) for i in range(nbuf)]

        @block.gpsimd
        def _(gpsimd):
            gpsimd.dma_start(idx_all[:], idx[:, :]).then_inc(sem_idx, 16)
            gpsimd.wait_ge(sem_idx, 16)
            for grp in range(ngrp):
                i, r = grp % nbuf, grp // nbuf
                if r >= 1:
                    # WAR: this buffer's previous-round store has completed.
                    gpsimd.wait_ge(sem_s[i], r * 16)
                for c in range(g_size):
                    d = grp * g_size + c
                    gpsimd.indirect_dma_start(
                        out=bufs[i][:, c * SUB_DIM : (c + 1) * SUB_DIM],
                        out_offset=None,
                        in_=table,
                        in_offset=bass.IndirectOffsetOnAxis(
                            ap=idx_all[:, d : d + 1], axis=0
                        ),
                    ).then_inc(sem_g[i], 16)

        @block.sync
        def _(sync):
            for grp in range(ngrp):
                i, r = grp % nbuf, grp // nbuf
                sync.wait_ge(sem_g[i], (r + 1) * g_size * 16)
                sync.dma_start(
                    out[:, grp * g_size * SUB_DIM : (grp + 1) * g_size * SUB_DIM],
                    bufs[i][:],
                ).then_inc(sem_s[i], 16)
            for i in range(nbuf):
                sync.wait_ge(sem_s[i], n_store[i] * 16)

    return nc


_NC = None


def _get_nc():
    global _NC
    if _NC is None:
        _NC = build_nc()
    return _NC


LAST_RESULTS = None  # BassKernelResults of the most recent run (for test.py)


def kernel(input_ids, table0, table1, _trace: bool = False):
    input_ids = np.asarray(input_ids)
    flat = input_ids.reshape(-1).astype(np.uint32)
    pairs = np.empty((TOK_TOTAL, 2), dtype=np.int32)
    pairs[:, 0] = _hash_ids(flat, SEED)
    pairs[:, 1] = _hash_ids(flat, SEED + 1) + np.int32(NUM_EMB)

    table = np.ascontiguousarray(
        np.concatenate([np.asarray(table0), np.asarray(table1)], axis=0),
        dtype=np.float32,
    )

    in_maps = []
    for c in range(N_CORES):
        pc = pairs[c * TOK_CORE : (c + 1) * TOK_CORE].reshape(P, TOK_PART * 2)
        in_maps.append({"idx": np.ascontiguousarray(pc), "table": table})

    global LAST_RESULTS
    LAST_RESULTS = run_bass_kernel_spmd(
        _get_nc(), in_maps, core_ids=list(range(N_CORES)), trace=_trace
    )
    out = np.concatenate(
        [r["out"].reshape(TOK_CORE, EMBED) for r in LAST_RESULTS.results], axis=0
    )
    return out.reshape(BATCH, SEQ, EMBED)
